# revision 21
# baseline (speedup 1.0000x reference)
"""Block attention (no softmax) Trainium2 Bass kernel, v2.10 (~49.0us).

Problem: x:[8,8192,128] -> q,k,v projections -> per-256-block attention with
a +/-255-row K/V window, NO softmax, -> out:[8,8192,128].

Key algebraic identity: with no softmax, (Q K^T * s) V == (Q * s) (K^T V).
Per window n, M_n = sum_{r in win(n)} k_r v_r^T is a [128,128] matrix; then
out_blk = (Q_blk * s) @ M_n.

Sharding: batch (8) across the 8 NeuronCores, data-parallel.

Structure (per core):
  * x ships HOST-TRANSPOSED as xT[128, 8192] f16 (pure layout marshalling,
    zero device math) and loads via 4 plain wide DMAs with 4KB-contiguous
    per-partition runs -- no PE transposes, no DVE transpose copies, no
    DMA-xbar mode serialization.
  * k/v chunks are sliced in PADDED window coordinates: chunk c holds orig
    rows [128c-255, 128c-127) at partitions [0:stored).  Window n = padded
    chunks 2n..2n+5, each consuming a PREFIX of stored partitions, so every
    window matmul lhsT starts at partition 0 (no zeroed-row workarounds;
    in-chunk placement is irrelevant for K^T V as long as k,v row-align).
  * out is written f16 in PARTITION-MAJOR DRAM layout [128, 64, 128] (4KB
    contiguous per partition per DMA); the host un-permutes + upcasts.
  * Engine budget: PE does projections + windows + out matmuls; DVE drains
    kv (bias TT fused) + half the out staging; ACT drains qT (bias+scale
    fused) + m2 + the other half of out staging; GpSimd only memsets
    (GPSIMD has no PSUM access on TRN2); Sync issues x/out DMAs, the
    Scalar HWDGE queue issues the const DMAs in parallel.
  * HAM warm-up: 12 dummy matmuls on a memset tile bridge PE activity
    from engine-init (~3us) to the first DMA-fed real work (~7.7us, after
    the fixed ~6.5us framework preamble), so the free-running 3.4us HAM
    activity window flips the PE clock 1.2->2.4 GHz during the fill phase
    instead of ~10us in.  Any PE gap >~0.5us before the flip re-arms the
    window (measured: each such gap costs ~3-4us of half-clock work), so
    the dummy count and the graduated first x-load (512 cols) are tuned to
    make the bridge gap-free.

Measured on 8xTRN2 (batch-parallel, per-core trace): 48.9-49.4us vs the
51.9us v1 baseline; rel err 5.3e-4 (fp16 operands, fp32 PSUM).  Further
known headroom: the ~6.5us engine-init preamble and ~3us teardown are
framework-fixed; steady-state Tensor occupancy is ~94%.
"""

import sys
from contextlib import ExitStack

import numpy as np

for _p in ("/opt/trn_rl_repo", "/root/.axon_site/_ro/trn_rl_repo"):
    if _p not in sys.path:
        sys.path.append(_p)

import concourse.bass as bass
import concourse.tile as tile
from concourse import bacc, mybir
from concourse.bass_utils import run_bass_kernel_spmd

S = 8192          # sequence length per batch/core
D = 128           # input dim
H = 128           # hidden dim
BS = 256          # block size
HALO = 255        # window_size - 1
NB = S // BS      # 32 blocks
NCORES = 8
SCALE = float(1.0 / np.sqrt(np.float32(D)))

F32 = mybir.dt.float32
F16 = mybir.dt.float16
CDT = F16
AF = mybir.ActivationFunctionType

WARMUP_MMS = 31    # dummy N=256 matmuls bridging engine-init -> first data
FILL_CIS = 4       # cis that get extra PE filler dummies (pipeline fill)

# emitter schedule: ci -> u group (delayed so the fill phase builds PE
# backlog before window matmuls start depending on the DVE bias-TT chain)
C2_AT = {5: 0, 6: 1, 7: 2, 9: 3, 11: 4, 13: 5, 15: 6}
D2_AT = {6: 0, 7: 1, 8: 2, 10: 3, 12: 4, 14: 5}


def _chunk_stored(c):
    """Stored row count of padded chunk c (orig rows [128c-255, 128c-127))."""
    return min(S, 128 * c - 127) - max(0, 128 * c - 255)


def _win_chunks(n):
    """(chunk, prefix_rows) pairs for window n = padded chunks 2n..2n+5.

    Window n covers padded rows [256n, 256n+766); chunk 2n+5 is clipped to
    its first 126 rows.  Stored rows are always a prefix from partition 0.
    """
    out = []
    for c in range(2 * n, 2 * n + 6):
        p = _chunk_stored(c)
        if c == 2 * n + 5:
            p = min(126, p)
        if p > 0:
            out.append((c, p))
    return out


def build_nc():
    nc = bacc.Bacc(
        "TRN2",
        target_bir_lowering=False,
        debug=False,
        enable_asserts=False,
        num_devices=NCORES,
    )

    xT = nc.dram_tensor("xT", [128, S], CDT, kind="ExternalInput").ap()
    cw = nc.dram_tensor("cw", [128, 3 * H], CDT, kind="ExternalInput").ap()
    cb = nc.dram_tensor("cb", [128, 4 * 2 * H], CDT, kind="ExternalInput").ap()
    cq = nc.dram_tensor("cq", [128, 1], F32, kind="ExternalInput").ap()
    # partition-major out: [p, c, h] with DRAM offset = (p*64 + c)*128 + h
    out = nc.dram_tensor("out", [128, NB * 2, H], CDT, kind="ExternalOutput").ap()

    with ExitStack() as ctx:
        tc = ctx.enter_context(tile.TileContext(nc))
        const = ctx.enter_context(tc.tile_pool(name="const", bufs=1))
        cw_sb = const.tile([128, 3 * H], CDT)
        wq_sb = cw_sb[:, 0:H]
        wkv_sb = cw_sb[:, H : 3 * H]
        bq_sb = const.tile([128, 1], F32)
        bkv4_sb = const.tile([128, 4, 2 * H], CDT)
        warm_sb = const.tile([128, 512], CDT)

        big = ctx.enter_context(tc.tile_pool(name="big", bufs=1))
        xT_all = big.tile([128, S], CDT)                 # x^T  [d, s]
        qT_all = big.tile([128, S], CDT)                 # q^T, scaled [h, s]
        kv_all = big.tile([128, 66, 2 * H], CDT)         # padded chunks [p,c,k|v]

        m_pool = ctx.enter_context(tc.tile_pool(name="m", bufs=2))
        o_pool = ctx.enter_context(tc.tile_pool(name="o", bufs=2))
        psum = ctx.enter_context(
            tc.tile_pool(name="ps", bufs=3, space=bass.MemorySpace.PSUM)
        )
        pskv = ctx.enter_context(
            tc.tile_pool(name="pskv", bufs=2, space=bass.MemorySpace.PSUM)
        )
        psw_pool = ctx.enter_context(
            tc.tile_pool(name="psw", bufs=1, space=bass.MemorySpace.PSUM)
        )

        # ---- warm-up feed + DMAs.  Consts go on the Scalar HWDGE queue so
        # they land in parallel with the first xT load on Sync.
        nc.gpsimd.memset(warm_sb, 0.0)
        nc.scalar.dma_start(cw_sb, cw)
        nc.scalar.dma_start(bkv4_sb, cb)
        nc.scalar.dma_start(bq_sb, cq)
        # graduated loads: small first chunk so ci0 can start ~1.5us sooner
        xl = 0
        for w in (512, 512, 1024, 2048, 4096):
            nc.sync.dma_start(
                xT_all[:, xl : xl + w], xT[:, xl : xl + w]
            )
            xl += w

        # ---- PE warm-up: HAM needs ~3.4us of sustained activity; dummies
        # run from engine-init (~3us) until real DMA-fed work arrives.
        psW = psw_pool.tile([128, 2, 256], F32, tag="psw", name="psW")
        _dummy_i = [0]

        def dummy_mm(k=1):
            for _ in range(k):
                nc.tensor.matmul(
                    psW[:, _dummy_i[0] % 2, :],
                    warm_sb[:, 0:128],
                    warm_sb[:, 0:256],
                    start=True,
                    stop=True,
                )
                _dummy_i[0] += 1

        dummy_mm(WARMUP_MMS)

        # ---- emitters ------------------------------------------------------
        def kv_chunk_mm(psKV, slot, c):
            if c == 1:
                # full-width: row r lands at partition r; windows only read
                # partition 0 (= orig row 0), the rest is benign garbage
                lhsT = xT_all[:, 0:128]
                dst = psKV[:, slot, :]
            else:
                a = 128 * c - 255
                p = _chunk_stored(c)
                lhsT = xT_all[:, a : a + p]
                dst = psKV[0:p, slot, :]
            nc.tensor.matmul(dst, lhsT, wkv_sb, start=True, stop=True)

        m2_tiles = {}

        def emit_c2(u):
            """K^T V for windows 4u..4u+3 -> psM4 (f32) -> m2 (f16)."""
            psM4 = psum.tile([128, 4, 128], F32, tag="ps", name="psM4")
            for w in range(4):
                n = 4 * u + w
                chunks = _win_chunks(n)
                for i, (c, p) in enumerate(chunks):
                    nc.tensor.matmul(
                        psM4[:, w, :],
                        kv_all[0:p, c, 0:H],
                        kv_all[0:p, c, H : 2 * H],
                        start=(i == 0),
                        stop=(i == len(chunks) - 1),
                    )
            m2 = m_pool.tile([128, 4, 128], CDT, tag="m")
            nc.scalar.copy(m2, psM4)
            m2_tiles[u] = m2

        ostage2 = {}

        def emit_d2(u, split_dma=False):
            """out rows [1024u, 1024u+1024) = (Q*s) @ M -> staging -> DRAM.

            Staging tiles span two u-groups; the DMA fires on the odd u.
            """
            m2 = m2_tiles.pop(u)
            if u % 2 == 0:
                ostage2[u // 2] = o_pool.tile(
                    [128, 16, 128], CDT, tag="o", name="ostage"
                )
            ostage = ostage2[u // 2]
            off = 8 * (u % 2)
            for half in range(2):
                psO = psum.tile([128, 4, 128], F32, tag="ps", name="psO")
                for w4 in range(4):
                    w = 4 * half + w4
                    n = 4 * u + w // 2
                    s0 = BS * n + 128 * (w % 2)
                    nc.tensor.matmul(
                        psO[:, w4, :],
                        qT_all[:, s0 : s0 + 128],
                        m2[:, w // 2, :],
                        start=True,
                        stop=True,
                    )
                dst = ostage[:, off + 4 * half : off + 4 * half + 4, :]
                if half == 0:
                    nc.scalar.copy(dst, psO)
                else:
                    nc.vector.tensor_copy(dst, psO)
            if u % 2 == 1:
                base = 16 * (u // 2)
                if split_dma:
                    nc.sync.dma_start(
                        out[:, base : base + 8, :], ostage[:, 0:8, :]
                    )
                    nc.sync.dma_start(
                        out[:, base + 8 : base + 16, :], ostage[:, 8:16, :]
                    )
                else:
                    nc.sync.dma_start(out[:, base : base + 16, :], ostage)
                del ostage2[u // 2]

        # ---- main loop: ci = 512-col stripe of xT --------------------------
        for ci in range(16):
            psKV = pskv.tile([128, 4, 2 * H], F32, tag="pskv", name="psKV")
            kv_chunk_mm(psKV, 0, 4 * ci + 1)
            kv_chunk_mm(psKV, 1, 4 * ci + 2)

            if ci in C2_AT:
                emit_c2(C2_AT[ci])
            if ci in D2_AT:
                emit_d2(D2_AT[ci])

            kv_chunk_mm(psKV, 2, 4 * ci + 3)
            kv_chunk_mm(psKV, 3, 4 * ci + 4)
            nc.vector.tensor_add(
                kv_all[:, 4 * ci + 1 : 4 * ci + 5, :], psKV, bkv4_sb
            )
            if ci < FILL_CIS:
                dummy_mm(1)

            # q^T stripe: [h, 512] = wq^T.T @ xT ; bias+scale fused on ACT
            psQ = psum.tile([128, 512], F32, tag="ps", name="psQ")
            nc.tensor.matmul(
                psQ,
                wq_sb,
                xT_all[:, 512 * ci : 512 * (ci + 1)],
                start=True,
                stop=True,
            )
            nc.scalar.activation(
                qT_all[:, 512 * ci : 512 * (ci + 1)],
                psQ,
                AF.Identity,
                bias=bq_sb,
                scale=SCALE,
            )

        # ---- epilogue: chunk 65, then the last window/out groups ----------
        psKV65 = psum.tile([128, 2 * H], F32, tag="ps", name="psKV65")
        p65 = _chunk_stored(65)  # 127
        nc.tensor.matmul(
            psKV65[0:p65, :], xT_all[:, 8065 : 8065 + p65], wkv_sb,
            start=True, stop=True,
        )
        nc.vector.tensor_add(
            kv_all[0:p65, 65, :], psKV65[0:p65, :], bkv4_sb[0:p65, 0, :]
        )

        emit_d2(6)
        emit_c2(7)
        emit_d2(7, split_dma=True)

    nc.compile()
    return nc


_NC_CACHE = None


def _get_nc():
    global _NC_CACHE
    if _NC_CACHE is None:
        _NC_CACHE = build_nc()
    return _NC_CACHE


def _make_in_maps(inputs):
    x = np.asarray(inputs["x"], dtype=np.float32)
    Wq = np.asarray(inputs["Wq"], dtype=np.float32)
    Wk = np.asarray(inputs["Wk"], dtype=np.float32)
    Wv = np.asarray(inputs["Wv"], dtype=np.float32)
    bq = np.asarray(inputs["bq"], dtype=np.float32)
    bk = np.asarray(inputs["bk"], dtype=np.float32)
    bv = np.asarray(inputs["bv"], dtype=np.float32)

    cw = np.concatenate([Wq.T, Wk.T, Wv.T], axis=1).astype(np.float16)
    # ACT computes func(in*scale + bias), so the q bias ships pre-scaled
    cq = (bq * SCALE).reshape(H, 1).astype(np.float32)
    cb = np.ascontiguousarray(
        np.broadcast_to(
            np.tile(np.concatenate([bk, bv]), 4)[None, :], (128, 8 * H)
        ).astype(np.float16)
    )

    shared = {
        "cw": np.ascontiguousarray(cw),
        "cb": cb,
        "cq": np.ascontiguousarray(cq),
    }
    x16 = x.astype(np.float16)
    return [
        {"xT": np.ascontiguousarray(x16[c].T), **shared} for c in range(NCORES)
    ]


def _collect(res):
    outs = []
    for c in range(NCORES):
        o = np.asarray(res.results[c]["out"])  # [128, 64, 128] p-major f16
        outs.append(np.ascontiguousarray(o.transpose(1, 0, 2)).reshape(S, H))
    return np.stack(outs, axis=0).astype(np.float32)


def kernel(**inputs):
    nc = _get_nc()
    in_maps = _make_in_maps(inputs)
    res = run_bass_kernel_spmd(nc, in_maps, core_ids=list(range(NCORES)))
    return _collect(res)


def run_traced(inputs):
    """Like kernel() but with NTFF tracing; returns (out, BassKernelResults)."""
    nc = _get_nc()
    in_maps = _make_in_maps(inputs)
    res = run_bass_kernel_spmd(
        nc, in_maps, core_ids=list(range(NCORES)), trace=True
    )
    return _collect(res), res


# revision 22
# speedup vs baseline: 1.0156x; 1.0156x over previous
"""Block attention (no softmax) Trainium2 Bass kernel, v2.10 (~49.0us).

Problem: x:[8,8192,128] -> q,k,v projections -> per-256-block attention with
a +/-255-row K/V window, NO softmax, -> out:[8,8192,128].

Key algebraic identity: with no softmax, (Q K^T * s) V == (Q * s) (K^T V).
Per window n, M_n = sum_{r in win(n)} k_r v_r^T is a [128,128] matrix; then
out_blk = (Q_blk * s) @ M_n.

Sharding: batch (8) across the 8 NeuronCores, data-parallel.

Structure (per core):
  * x ships HOST-TRANSPOSED as xT[128, 8192] f16 (pure layout marshalling,
    zero device math) and loads via 4 plain wide DMAs with 4KB-contiguous
    per-partition runs -- no PE transposes, no DVE transpose copies, no
    DMA-xbar mode serialization.
  * k/v chunks are sliced in PADDED window coordinates: chunk c holds orig
    rows [128c-255, 128c-127) at partitions [0:stored).  Window n = padded
    chunks 2n..2n+5, each consuming a PREFIX of stored partitions, so every
    window matmul lhsT starts at partition 0 (no zeroed-row workarounds;
    in-chunk placement is irrelevant for K^T V as long as k,v row-align).
  * out is written f16 in PARTITION-MAJOR DRAM layout [128, 64, 128] (4KB
    contiguous per partition per DMA); the host un-permutes + upcasts.
  * Engine budget: PE does projections + windows + out matmuls; DVE drains
    kv (bias TT fused) + half the out staging; ACT drains qT (bias+scale
    fused) + m2 + the other half of out staging; GpSimd only memsets
    (GPSIMD has no PSUM access on TRN2); Sync issues x/out DMAs, the
    Scalar HWDGE queue issues the const DMAs in parallel.
  * HAM warm-up: 12 dummy matmuls on a memset tile bridge PE activity
    from engine-init (~3us) to the first DMA-fed real work (~7.7us, after
    the fixed ~6.5us framework preamble), so the free-running 3.4us HAM
    activity window flips the PE clock 1.2->2.4 GHz during the fill phase
    instead of ~10us in.  Any PE gap >~0.5us before the flip re-arms the
    window (measured: each such gap costs ~3-4us of half-clock work), so
    the dummy count and the graduated first x-load (512 cols) are tuned to
    make the bridge gap-free.

Measured on 8xTRN2 (batch-parallel, per-core trace): 48.9-49.4us vs the
51.9us v1 baseline; rel err 5.3e-4 (fp16 operands, fp32 PSUM).  Further
known headroom: the ~6.5us engine-init preamble and ~3us teardown are
framework-fixed; steady-state Tensor occupancy is ~94%.
"""

import sys
from contextlib import ExitStack

import numpy as np

for _p in ("/opt/trn_rl_repo", "/root/.axon_site/_ro/trn_rl_repo"):
    if _p not in sys.path:
        sys.path.append(_p)

import concourse.bass as bass
import concourse.tile as tile
from concourse import bacc, mybir
from concourse.bass_utils import run_bass_kernel_spmd

S = 8192          # sequence length per batch/core
D = 128           # input dim
H = 128           # hidden dim
BS = 256          # block size
HALO = 255        # window_size - 1
NB = S // BS      # 32 blocks
NCORES = 8
SCALE = float(1.0 / np.sqrt(np.float32(D)))

F32 = mybir.dt.float32
F16 = mybir.dt.float16
CDT = F16
AF = mybir.ActivationFunctionType

WARMUP_MMS = 26    # dummy N=256 matmuls bridging engine-init -> first data
FILL_CIS = 4       # cis that get extra PE filler dummies (pipeline fill)

# emitter schedule: ci -> u group (delayed so the fill phase builds PE
# backlog before window matmuls start depending on the DVE bias-TT chain)
C2_AT = {5: 0, 6: 1, 7: 2, 9: 3, 11: 4, 13: 5, 15: 6}
D2_AT = {6: 0, 7: 1, 8: 2, 10: 3, 12: 4, 14: 5}


def _chunk_stored(c):
    """Stored row count of padded chunk c (orig rows [128c-255, 128c-127))."""
    return min(S, 128 * c - 127) - max(0, 128 * c - 255)


def _win_chunks(n):
    """(chunk, prefix_rows) pairs for window n = padded chunks 2n..2n+5.

    Window n covers padded rows [256n, 256n+766); chunk 2n+5 is clipped to
    its first 126 rows.  Stored rows are always a prefix from partition 0.
    """
    out = []
    for c in range(2 * n, 2 * n + 6):
        p = _chunk_stored(c)
        if c == 2 * n + 5:
            p = min(126, p)
        if p > 0:
            out.append((c, p))
    return out


def build_nc():
    nc = bacc.Bacc(
        "TRN2",
        target_bir_lowering=False,
        debug=False,
        enable_asserts=False,
        num_devices=NCORES,
    )

    xT = nc.dram_tensor("xT", [128, S], CDT, kind="ExternalInput").ap()
    cw = nc.dram_tensor("cw", [128, 3 * H], CDT, kind="ExternalInput").ap()
    cb = nc.dram_tensor("cb", [128, 4 * 2 * H], CDT, kind="ExternalInput").ap()
    cq = nc.dram_tensor("cq", [128, 1], F32, kind="ExternalInput").ap()
    # partition-major out: [p, c, h] with DRAM offset = (p*64 + c)*128 + h
    out = nc.dram_tensor("out", [128, NB * 2, H], CDT, kind="ExternalOutput").ap()

    with ExitStack() as ctx:
        tc = ctx.enter_context(tile.TileContext(nc))
        const = ctx.enter_context(tc.tile_pool(name="const", bufs=1))
        cw_sb = const.tile([128, 3 * H], CDT)
        wq_sb = cw_sb[:, 0:H]
        wkv_sb = cw_sb[:, H : 3 * H]
        bq_sb = const.tile([128, 1], F32)
        bkv4_sb = const.tile([128, 4, 2 * H], CDT)
        warm_sb = const.tile([128, 512], CDT)

        big = ctx.enter_context(tc.tile_pool(name="big", bufs=1))
        xT_all = big.tile([128, S], CDT)                 # x^T  [d, s]
        qT_all = big.tile([128, S], CDT)                 # q^T, scaled [h, s]
        kv_all = big.tile([128, 66, 2 * H], CDT)         # padded chunks [p,c,k|v]

        m_pool = ctx.enter_context(tc.tile_pool(name="m", bufs=2))
        o_pool = ctx.enter_context(tc.tile_pool(name="o", bufs=3))
        psum = ctx.enter_context(
            tc.tile_pool(name="ps", bufs=3, space=bass.MemorySpace.PSUM)
        )
        pskv = ctx.enter_context(
            tc.tile_pool(name="pskv", bufs=2, space=bass.MemorySpace.PSUM)
        )
        psw_pool = ctx.enter_context(
            tc.tile_pool(name="psw", bufs=1, space=bass.MemorySpace.PSUM)
        )

        # ---- warm-up feed + DMAs.  Consts go on the Scalar HWDGE queue so
        # they land in parallel with the first xT load on Sync.
        nc.gpsimd.memset(warm_sb, 0.0)
        nc.scalar.dma_start(cw_sb, cw)
        nc.scalar.dma_start(bkv4_sb, cb)
        nc.scalar.dma_start(bq_sb, cq)
        # graduated loads: small first chunk so ci0 can start ~1.5us sooner
        xl = 0
        for w in (512, 512, 1024, 2048, 4096):
            nc.sync.dma_start(
                xT_all[:, xl : xl + w], xT[:, xl : xl + w]
            )
            xl += w

        # ---- PE warm-up: HAM needs ~3.4us of sustained activity; dummies
        # run from engine-init (~3us) until real DMA-fed work arrives.
        psW = psw_pool.tile([128, 2, 256], F32, tag="psw", name="psW")
        _dummy_i = [0]

        def dummy_mm(k=1):
            for _ in range(k):
                nc.tensor.matmul(
                    psW[:, _dummy_i[0] % 2, :],
                    warm_sb[:, 0:128],
                    warm_sb[:, 0:256],
                    start=True,
                    stop=True,
                )
                _dummy_i[0] += 1

        dummy_mm(WARMUP_MMS)

        # ---- emitters ------------------------------------------------------
        def kv_chunk_mm(psKV, slot, c):
            if c == 1:
                # full-width: row r lands at partition r; windows only read
                # partition 0 (= orig row 0), the rest is benign garbage
                lhsT = xT_all[:, 0:128]
                dst = psKV[:, slot, :]
            else:
                a = 128 * c - 255
                p = _chunk_stored(c)
                lhsT = xT_all[:, a : a + p]
                dst = psKV[0:p, slot, :]
            nc.tensor.matmul(dst, lhsT, wkv_sb, start=True, stop=True)

        m2_tiles = {}

        def emit_c2(u):
            """K^T V for windows 4u..4u+3 -> psM4 (f32) -> m2 (f16)."""
            psM4 = psum.tile([128, 4, 128], F32, tag="ps", name="psM4")
            for w in range(4):
                n = 4 * u + w
                chunks = _win_chunks(n)
                for i, (c, p) in enumerate(chunks):
                    nc.tensor.matmul(
                        psM4[:, w, :],
                        kv_all[0:p, c, 0:H],
                        kv_all[0:p, c, H : 2 * H],
                        start=(i == 0),
                        stop=(i == len(chunks) - 1),
                    )
            m2 = m_pool.tile([128, 4, 128], CDT, tag="m")
            nc.scalar.copy(m2, psM4)
            m2_tiles[u] = m2

        ostage2 = {}

        def emit_d2(u, split_dma=False):
            """out rows [1024u, 1024u+1024) = (Q*s) @ M -> staging -> DRAM.

            Staging tiles span two u-groups; the DMA fires on the odd u.
            """
            m2 = m2_tiles.pop(u)
            solo = u >= 6
            if solo:
                ostage = o_pool.tile([128, 8, 128], CDT, tag="o", name="ostage")
                off = 0
            else:
                if u % 2 == 0:
                    ostage2[u // 2] = o_pool.tile(
                        [128, 16, 128], CDT, tag="o", name="ostage"
                    )
                ostage = ostage2[u // 2]
                off = 8 * (u % 2)
            for half in range(2):
                psO = psum.tile([128, 4, 128], F32, tag="ps", name="psO")
                for w4 in range(4):
                    w = 4 * half + w4
                    n = 4 * u + w // 2
                    s0 = BS * n + 128 * (w % 2)
                    nc.tensor.matmul(
                        psO[:, w4, :],
                        qT_all[:, s0 : s0 + 128],
                        m2[:, w // 2, :],
                        start=True,
                        stop=True,
                    )
                dst = ostage[:, off + 4 * half : off + 4 * half + 4, :]
                if half == 0:
                    nc.scalar.copy(dst, psO)
                else:
                    nc.vector.tensor_copy(dst, psO)
            if solo:
                base = 8 * u
                if split_dma:
                    nc.sync.dma_start(
                        out[:, base : base + 4, :], ostage[:, 0:4, :]
                    )
                    nc.sync.dma_start(
                        out[:, base + 4 : base + 8, :], ostage[:, 4:8, :]
                    )
                else:
                    nc.sync.dma_start(out[:, base : base + 8, :], ostage)
            elif u % 2 == 1:
                base = 16 * (u // 2)
                nc.sync.dma_start(out[:, base : base + 16, :], ostage)
                del ostage2[u // 2]

        # ---- main loop: ci = 512-col stripe of xT --------------------------
        for ci in range(16):
            psKV = pskv.tile([128, 4, 2 * H], F32, tag="pskv", name="psKV")
            kv_chunk_mm(psKV, 0, 4 * ci + 1)
            kv_chunk_mm(psKV, 1, 4 * ci + 2)

            if ci in C2_AT:
                emit_c2(C2_AT[ci])
            if ci in D2_AT:
                emit_d2(D2_AT[ci])

            kv_chunk_mm(psKV, 2, 4 * ci + 3)
            kv_chunk_mm(psKV, 3, 4 * ci + 4)
            nc.vector.tensor_add(
                kv_all[:, 4 * ci + 1 : 4 * ci + 5, :], psKV, bkv4_sb
            )
            if ci < FILL_CIS:
                dummy_mm(1)

            # q^T stripe: [h, 512] = wq^T.T @ xT ; bias+scale fused on ACT
            psQ = psum.tile([128, 512], F32, tag="ps", name="psQ")
            nc.tensor.matmul(
                psQ,
                wq_sb,
                xT_all[:, 512 * ci : 512 * (ci + 1)],
                start=True,
                stop=True,
            )
            nc.scalar.activation(
                qT_all[:, 512 * ci : 512 * (ci + 1)],
                psQ,
                AF.Identity,
                bias=bq_sb,
                scale=SCALE,
            )

        # ---- epilogue: chunk 65, then the last window/out groups ----------
        psKV65 = psum.tile([128, 2 * H], F32, tag="ps", name="psKV65")
        p65 = _chunk_stored(65)  # 127
        nc.tensor.matmul(
            psKV65[0:p65, :], xT_all[:, 8065 : 8065 + p65], wkv_sb,
            start=True, stop=True,
        )
        nc.vector.tensor_add(
            kv_all[0:p65, 65, :], psKV65[0:p65, :], bkv4_sb[0:p65, 0, :]
        )

        emit_d2(6)
        emit_c2(7)
        emit_d2(7, split_dma=True)

    nc.compile()
    return nc


_NC_CACHE = None


def _get_nc():
    global _NC_CACHE
    if _NC_CACHE is None:
        _NC_CACHE = build_nc()
    return _NC_CACHE


def _make_in_maps(inputs):
    x = np.asarray(inputs["x"], dtype=np.float32)
    Wq = np.asarray(inputs["Wq"], dtype=np.float32)
    Wk = np.asarray(inputs["Wk"], dtype=np.float32)
    Wv = np.asarray(inputs["Wv"], dtype=np.float32)
    bq = np.asarray(inputs["bq"], dtype=np.float32)
    bk = np.asarray(inputs["bk"], dtype=np.float32)
    bv = np.asarray(inputs["bv"], dtype=np.float32)

    cw = np.concatenate([Wq.T, Wk.T, Wv.T], axis=1).astype(np.float16)
    # ACT computes func(in*scale + bias), so the q bias ships pre-scaled
    cq = (bq * SCALE).reshape(H, 1).astype(np.float32)
    cb = np.ascontiguousarray(
        np.broadcast_to(
            np.tile(np.concatenate([bk, bv]), 4)[None, :], (128, 8 * H)
        ).astype(np.float16)
    )

    shared = {
        "cw": np.ascontiguousarray(cw),
        "cb": cb,
        "cq": np.ascontiguousarray(cq),
    }
    x16 = x.astype(np.float16)
    return [
        {"xT": np.ascontiguousarray(x16[c].T), **shared} for c in range(NCORES)
    ]


def _collect(res):
    outs = []
    for c in range(NCORES):
        o = np.asarray(res.results[c]["out"])  # [128, 64, 128] p-major f16
        outs.append(np.ascontiguousarray(o.transpose(1, 0, 2)).reshape(S, H))
    return np.stack(outs, axis=0).astype(np.float32)


def kernel(**inputs):
    nc = _get_nc()
    in_maps = _make_in_maps(inputs)
    res = run_bass_kernel_spmd(nc, in_maps, core_ids=list(range(NCORES)))
    return _collect(res)


def run_traced(inputs):
    """Like kernel() but with NTFF tracing; returns (out, BassKernelResults)."""
    nc = _get_nc()
    in_maps = _make_in_maps(inputs)
    res = run_bass_kernel_spmd(
        nc, in_maps, core_ids=list(range(NCORES)), trace=True
    )
    return _collect(res), res
